# revision 13
# baseline (speedup 1.0000x reference)
"""Levina-Bickel MLE intrinsic-dimension kernel for Trainium2 (8 NeuronCores).

Problem: X [B=4, N=8192, D=32] f32, k=16.
  d2[b,i,j] = |x_i - x_j|^2 ; per row take 16 smallest (incl. self), drop self,
  s_i = sum_j log(d_16/d_j), out[b] = 14*N / sum_i s_i.

Sharding: core c -> batch c//2, query rows (c%2)*4096 ..+4096, full X[b]
replicated as the matmul moving operand.

Per core:
  key[i,j] = q_i . x_j - 0.5*|x_j|^2  (ranking by largest key == smallest d2)
  computed as ONE K=99 bf16 matmul per block: partitions 0-31 hold q_hi/x_hi,
  32-33 the (1, -0.5|x|^2) hi/lo norm rows, 34-65 q_hi/x_lo, 66-97 q_lo/x_hi,
  and row 98 a constant C=1024 that shifts every key positive (so the rank-1..8
  mask in the merge can use multiply-by-0).  PE cost depends only on the moving
  dim, so the whole split is free -> near-fp32 accuracy at 1 bf16-matmul cost.
  Top-16 per row: DVE max8 per 1024-col block (8 blocks) -> 64 candidates,
  then max8 + match_replace + max8 to get the 16 largest keys (= 16 smallest
  d2, ascending).  ACT computes L = ln(sq_i - 2*key) with per-partition bias
  and a fused free-dim accumulate; DVE folds s'_i = 15*L_16 - sum(L).
  Host sums the per-core [128, 32] partials and finishes the scalar math.
"""

import sys

sys.path.insert(0, "/opt/trn_rl_repo")

import numpy as np
import ml_dtypes

import concourse.bass as bass  # noqa: F401  (registers bass types)
import concourse.bacc as bacc
import concourse.tile as tile
import concourse.mybir as mybir
from concourse.bass_utils import run_bass_kernel_spmd

BF16 = ml_dtypes.bfloat16
B, N, D, KNN = 4, 8192, 32, 16
NCORES = 8
ROWS_PER_CORE = B * N // NCORES      # 4096
TILES = ROWS_PER_CORE // 128         # 32
NBLK = 8
BLK = N // NBLK                      # 1024
KEY_SHIFT = 1024.0  # d2 = (sq_i + 2C) - 2*key'

_compiled = None


def _build():
    nc = bacc.Bacc("TRN2", target_bir_lowering=False, debug=False)
    f32 = mybir.dt.float32
    bf16 = mybir.dt.bfloat16

    xt_d = nc.dram_tensor("xt", [128, N], bf16, kind="ExternalInput")
    qt_d = nc.dram_tensor("qt", [128, ROWS_PER_CORE], bf16, kind="ExternalInput")
    sq_d = nc.dram_tensor("sqq", [128, TILES], f32, kind="ExternalInput")
    out_d = nc.dram_tensor("acc_out", [128, TILES], f32, kind="ExternalOutput")

    with tile.TileContext(nc) as tc:
        with (
            tc.tile_pool(name="persist", bufs=1) as persist,
            tc.tile_pool(name="psum", bufs=4, space="PSUM") as psum_pool,
            tc.tile_pool(name="work", bufs=4) as work,
        ):
            xt = persist.tile([128, N], bf16)
            qt = persist.tile([128, ROWS_PER_CORE], bf16)
            sqq = persist.tile([128, TILES], f32)
            acc = persist.tile([128, TILES], f32)

            nc.sync.dma_start(qt[:], qt_d.ap()[:])
            for blk in range(NBLK):
                nc.sync.dma_start(xt[:, blk * BLK : (blk + 1) * BLK],
                                  xt_d.ap()[:, blk * BLK : (blk + 1) * BLK])
            nc.sync.dma_start(sqq[:], sq_d.ap()[:])

            # PE HAM warmup: ~7us of dep-free back-to-back matmuls on scratch
            # data so the tensor engine reaches its fast clock before (and
            # overlapping with) the input DMA.
            dummy = persist.tile([128, 512], bf16)
            nc.gpsimd.memset(dummy[:], 0.0)
            for _ in range(14):
                ps_w = psum_pool.tile([128, BLK], f32, tag="ps", name="ps_warm")
                nc.tensor.matmul(ps_w[:, 0:512], dummy[0:98, 0:128],
                                 dummy[0:98, :], start=True, stop=True)

            def merge(t, cands):
                """Top-16 of the 64 block candidates + MLE fold for tile t."""
                sel = work.tile([128, 16], f32, tag="sel", name="sel")
                cands2 = work.tile([128, NBLK * 8], f32, tag="cands2", name="cands2")
                nc.vector.max(sel[:, 0:8], cands[:])
                # keys are > 0 (C shift), so masking ranks 1-8 to 0 drops them
                nc.vector.scalar_tensor_tensor(
                    cands2[:], cands[:], sel[:, 7:8], cands[:],
                    op0=mybir.AluOpType.is_lt, op1=mybir.AluOpType.mult,
                )
                nc.vector.max(sel[:, 8:16], cands2[:])

                logs = work.tile([128, KNN - 1], f32, tag="logs", name="logs")
                r = work.tile([128, 1], f32, tag="r", name="r")
                nc.scalar.activation(
                    logs[:], sel[:, 1:16], mybir.ActivationFunctionType.Ln,
                    bias=sqq[:, t : t + 1], scale=-2.0, accum_out=r[:],
                )
                # s' = 15*L_16 - sum(L), as two tiny ACT ops (Identity lives in
                # the same HW act table as Ln) so the DVE stream never waits.
                t15 = work.tile([128, 1], f32, tag="t15", name="t15")
                nc.scalar.activation(
                    t15[:], logs[:, KNN - 2 : KNN - 1],
                    mybir.ActivationFunctionType.Identity, scale=float(KNN - 1),
                )
                nc.scalar.activation(
                    acc[:, t : t + 1], r[:],
                    mybir.ActivationFunctionType.Identity, bias=t15[:], scale=-1.0,
                )

            # Software-pipelined: tile t's merge is emitted after tile t+1's
            # block max8s, so its dependencies are ~9us stale when the DVE
            # reaches it and the PE gets slack to run ahead.
            pending = None
            for t in range(TILES):
                w = qt[:, t * 128 : (t + 1) * 128]
                cands = work.tile([128, NBLK * 8], f32, tag="cands", name="cands")
                for blk in range(NBLK):
                    ps = psum_pool.tile([128, BLK], f32, tag="ps", name="ps")
                    for h in range(BLK // 512):
                        c0 = blk * BLK + h * 512
                        o = ps[:, h * 512 : (h + 1) * 512]
                        x = xt[:, c0 : c0 + 512]
                        nc.tensor.matmul(o, w[0:99, :], x[0:99, :],
                                         start=True, stop=True)
                    nc.vector.max(cands[:, blk * 8 : (blk + 1) * 8], ps[:])
                if pending is not None:
                    merge(*pending)
                pending = (t, cands)
            merge(*pending)

            nc.sync.dma_start(out_d.ap()[:], acc[:])

    nc.compile()
    return nc


def get_compiled():
    global _compiled
    if _compiled is None:
        _compiled = _build()
    return _compiled


def _split(a):
    hi = a.astype(BF16)
    lo = (a - hi.astype(np.float32)).astype(BF16)
    return hi, lo


def prep_inputs(X):
    """X [B, N, D] f32 -> per-core input maps + per-query |q|^2 table."""
    in_maps = []
    for c in range(NCORES):
        b, h = c // 2, c % 2
        Xb = np.ascontiguousarray(X[b])                       # [N, D] f32
        sqx = (Xb.astype(np.float64) ** 2).sum(1)             # [N] f64
        x33 = (-0.5 * sqx).astype(np.float32)
        Xhi, Xlo = _split(Xb)
        x33hi, x33lo = _split(x33)

        xt = np.zeros([128, N], BF16)
        xt[0:32] = Xhi.T
        xt[32] = x33hi
        xt[33] = x33lo
        xt[34:66] = Xlo.T
        xt[66:98] = Xhi.T
        xt[98] = BF16(KEY_SHIFT)

        Qb = Xb[h * ROWS_PER_CORE : (h + 1) * ROWS_PER_CORE]  # [4096, D]
        Qhi, Qlo = _split(Qb)
        qt = np.zeros([128, ROWS_PER_CORE], BF16)
        qt[0:32] = Qhi.T
        qt[32] = BF16(1.0)
        qt[33] = BF16(1.0)
        qt[34:66] = Qhi.T
        qt[66:98] = Qlo.T
        qt[98] = BF16(1.0)

        sq_core = (sqx[h * ROWS_PER_CORE : (h + 1) * ROWS_PER_CORE]
                   + 2.0 * KEY_SHIFT).astype(np.float32)
        sqq = np.ascontiguousarray(sq_core.reshape(TILES, 128).T)  # [128, TILES]

        in_maps.append({"xt": xt, "qt": qt, "sqq": sqq})
    return in_maps


def finish(acc_list):
    """acc_list: per-core [128, TILES] f32 of s'_i = 2*s_i. -> out [B] f32."""
    S = np.zeros(B, np.float64)
    for c, a in enumerate(acc_list):
        S[c // 2] += a.astype(np.float64).sum()
    # out_b = (k-2)*N / sum_i s_i  with  sum s_i = 0.5 * S_b
    return (2.0 * (KNN - 2) * N / S).astype(np.float32)


def kernel(X, k):
    assert int(k) == KNN
    X = np.asarray(X, dtype=np.float32)
    assert X.shape == (B, N, D)
    nc = get_compiled()
    in_maps = prep_inputs(X)
    res = run_bass_kernel_spmd(nc, in_maps, list(range(NCORES)))
    acc_list = [res.results[c]["acc_out"] for c in range(NCORES)]
    return finish(acc_list)
